# revision 1
# baseline (speedup 1.0000x reference)
"""BrahmaAttention (GQA prefill with KV cache) on 8 Trainium2 NeuronCores.

Problem: B=4, S=1024, C=1024 (cache), H=16 q-heads, G=4 kv-heads, D=128.
    q = hs @ wq.T ; k = hs @ wk.T ; v = hs @ wv.T
    rope(q, k) (interleaved pairs, positions C..C+S)
    k_full/v_full = concat(cache, new)           # K = 2048 keys
    out = softmax(q k^T / sqrt(D)) @ v_full @ wo.T
(attention_mask is all-zeros by construction - full attention, no masking.)

Sharding: 4-way data parallel over batch x 2-way tensor parallel over heads.
core (b, hg) handles batch b, q-heads hg*8..hg*8+8, kv-heads hg*2..hg*2+2 and
computes a partial output projection over its 1024 hidden columns; the host
sums the two partials per batch (the TP all-reduce done on host at gather).

Host-side prep folded into the shards:
  - 1/sqrt(D) folded into wq.
  - RoPE even/odd interleave permuted to [evens|odds] via wq/wk row
    permutation and cache_k last-dim permutation, so on-chip RoPE is
    half-tile elementwise ops (partitions 0-63 = even, 64-127 = odd lanes).
  - All projection weights pre-transposed/tiled so every DMA is contiguous
    and every matmul operand is naturally [K=128, *] in SBUF.

On-chip layout per core: everything transposed, [D, S]-style:
  qT [128, 8, 1024]  kT_full [128, 2, 2048]  v_full [128, 16, 2, 128]
  scoresT = kT.T @ qT per head -> [keys, s] in PSUM -> exp on ACT -> probs
  denominator: ones[128,128].T @ probs accumulated on PE, reciprocal via
  DVE approx, broadcast back over partitions with a K=1 matmul, and the
  normalization is fused into the AV PSUM->SBUF copyback.
"""

import numpy as np

B, S, C, H, G, D = 4, 1024, 1024, 16, 4, 128
HID = H * D
P = 128
NH, NG = 8, 2          # per-core q heads / kv heads
KC = (C + S) // P      # 16 key chunks
KT = 16                # hid contraction tiles
SH = 512               # s-half (PSUM bank free size)
N_CORES = 8

_PERM = np.concatenate([np.arange(0, D, 2), np.arange(1, D, 2)])

_BUILT = {}
_MM_F32R = True


def _mm(nc, out, lhsT, rhs, **kw):
    """All matmul operands live in float32r tiles (TF32-like: fp32 storage,
    rounded mantissa, 4x the PE streaming rate of full fp32)."""
    nc.tensor.matmul(out, lhsT, rhs, **kw)



def _rope(nc, pool, f32, psum_in, out_ap, cs_cc, cs_pm, mult):
    """out = psum_in*[cos;cos] + swap_halves(psum_in*[sin;-sin]).

    psum_in is the raw projected [128, S] tile with evens on partitions 0-63
    and odds on 64-127; out gets the roped value in the same layout.
    """
    import concourse.mybir as mybir

    a = pool.tile([P, S], f32, tag="ropeA", name="ropeA")
    b = pool.tile([P, S], f32, tag="ropeB", name="ropeB")
    s = pool.tile([P, S], f32, tag="ropeS", name="ropeS")
    nc.vector.tensor_tensor(a[:], psum_in[:], cs_cc[:], mult)
    nc.vector.tensor_tensor(b[:], psum_in[:], cs_pm[:], mult)
    # swap halves on the scalar engine (idle during phase 1)
    nc.scalar.copy(s[0:64, :], b[64:128, :])
    nc.scalar.copy(s[64:128, :], b[0:64, :])
    nc.vector.tensor_tensor(out_ap, a[:], s[:], mybir.AluOpType.add)


def build_bass(unroll=1):
    """Build + compile the per-core Bass program (identical on all cores)."""
    if unroll in _BUILT:
        return _BUILT[unroll]

    import concourse.mybir as mybir
    import concourse.tile as tile
    from concourse import bacc

    f32 = mybir.dt.float32
    f32r = mybir.dt.float32r if _MM_F32R else mybir.dt.float32
    mult = mybir.AluOpType.mult
    add = mybir.AluOpType.add
    Exp = mybir.ActivationFunctionType.Exp

    nc = bacc.Bacc("TRN2", target_bir_lowering=False, debug=False)

    hsT_d = nc.dram_tensor("hsT", [KT, P, S], f32r, kind="ExternalInput")
    wq_d = nc.dram_tensor("wqT", [NH, KT, P, P], f32r, kind="ExternalInput")
    wk_d = nc.dram_tensor("wkT", [NG, KT, P, P], f32r, kind="ExternalInput")
    wv_d = nc.dram_tensor("wvT", [KT, P, NG * P], f32r, kind="ExternalInput")
    wo_d = nc.dram_tensor("woT", [NH, P, HID], f32r, kind="ExternalInput")
    ck_d = nc.dram_tensor("ckT", [NG, P, C], f32r, kind="ExternalInput")
    cv_d = nc.dram_tensor("cvP", [P, C // P, NG, P], f32r, kind="ExternalInput")
    cc_d = nc.dram_tensor("cs_cc", [P, S], f32, kind="ExternalInput")
    pm_d = nc.dram_tensor("cs_pm", [P, S], f32, kind="ExternalInput")
    y_d = nc.dram_tensor("y", [S, HID], f32, kind="ExternalOutput")

    with tile.TileContext(nc) as tc:
        with tc.tile_pool(name="const", bufs=1) as const:
            ones_f = const.tile([P, P], f32, name="ones_f")
            nc.any.memset(ones_f[:], 1.0)
            ones128 = const.tile([P, P], f32r, name="ones128")
            nc.vector.tensor_copy(ones128[:], ones_f[:])
            ones1 = const.tile([1, P], f32r, name="ones1")
            nc.vector.tensor_copy(ones1[:], ones_f[0:1, :])
            cs_cc = const.tile([P, S], f32, name="cs_cc")
            cs_pm = const.tile([P, S], f32, name="cs_pm")

            for it in range(unroll):
                _emit_iteration(
                    nc, tc, f32, f32r, mult, add, Exp,
                    hsT_d, wq_d, wk_d, wv_d, wo_d, ck_d, cv_d, y_d,
                    ones128, ones1, cs_cc, cs_pm,
                    cs_load=(cc_d, pm_d) if it == 0 else None,
                )

    nc.compile()
    _BUILT[unroll] = nc
    return nc


def _emit_iteration(nc, tc, f32, f32r, mult, add, Exp,
                    hsT_d, wq_d, wk_d, wv_d, wo_d, ck_d, cv_d, y_d,
                    ones128, ones1, cs_cc, cs_pm, cs_load=None):
    import concourse.tile as tile  # noqa: F401

    with tc.tile_pool(name="persist", bufs=1) as persist:
        qT = persist.tile([P, NH, S], f32r, name="qT")
        kT = persist.tile([P, NG, C + S], f32r, name="kT")
        vF = persist.tile([P, KC, NG, P], f32r, name="vF")

        # ---------------- phase 1: projections + rope ----------------
        with (
            tc.tile_pool(name="ph1", bufs=1) as ph1,
            tc.tile_pool(name="wq_pool", bufs=2) as wq_pool,
            tc.tile_pool(name="wk_pool", bufs=2) as wk_pool,
            tc.tile_pool(name="rope", bufs=1) as rope_pool,
            tc.tile_pool(name="ps1", bufs=1, space="PSUM") as ps1,
        ):
            if cs_load is not None:
                # PE/HAM warm-up during the initial DMA window (iteration 0
                # only): ~130 back-to-back tiny matmuls, consumed by one copy
                # so they survive DCE.
                pw = ps1.tile([P, P], f32, tag="warm", bufs=1, name="pwarm")
                for i in range(150):
                    _mm(nc, pw[:], ones128[:], ones128[:],
                        start=(i == 0), stop=(i == 149), skip_group_check=True)
                wsink = rope_pool.tile([1, 1], f32, tag="wsink", name="wsink")
                nc.vector.tensor_copy(wsink[:], pw[0:1, 0:1])
            # DMA emission order = queue order: first matmul needs wk half 0
            # + hsT chunk 0, so those go first; bulk loads (cache, wv) that
            # are only needed later in the phase go last.
            wks = []
            hsT = ph1.tile([P, KT, S], f32r, name="hsT_sb")
            for g in range(NG):
                wk = wk_pool.tile([P, KT, P], f32r, tag="wk", name="wk_sb")
                nc.sync.dma_start(wk[:, 0:KT // 2, :],
                                  wk_d[g, 0:KT // 2].rearrange("k p m -> p k m"))
                wks.append(wk)
                if g == 0:
                    nc.sync.dma_start(
                        hsT[:, 0:2, :], hsT_d[0:2].rearrange("k p s -> p k s"))
            for g in range(NG):
                nc.sync.dma_start(wks[g][:, KT // 2:, :],
                                  wk_d[g, KT // 2:].rearrange("k p m -> p k m"))
            if cs_load is not None:
                nc.sync.dma_start(cs_cc[:], cs_load[0][:])
                nc.sync.dma_start(cs_pm[:], cs_load[1][:])
            for i in range(1, 8):
                nc.sync.dma_start(
                    hsT[:, 2 * i:2 * i + 2, :],
                    hsT_d[2 * i:2 * i + 2].rearrange("k p s -> p k s"),
                )
            # q weights for the first heads ahead of the bulk cache loads
            wqs = {}
            for h in range(2):
                wq = wq_pool.tile([P, KT, P], f32r, tag="wq", name="wq_sb")
                nc.sync.dma_start(wq[:], wq_d[h].rearrange("k p m -> p k m"))
                wqs[h] = wq
            for g in range(NG):
                nc.sync.dma_start(kT[:, g, 0:C], ck_d[g])
            nc.sync.dma_start(vF[:, 0:C // P, :, :], cv_d[:])
            wv = ph1.tile([P, KT, NG * P], f32r, name="wv_sb")
            nc.sync.dma_start(wv[:], wv_d.rearrange("k p n -> p k n"))

            # k projection + rope (new keys go to kT[:, g, C:])
            for g in range(NG):
                wk = wks[g]
                pk = ps1.tile([P, S], f32, tag="pqk", bufs=2, name="pk")
                for k in range(KT):
                    for n in range(2):
                        _mm(
    nc,
                            pk[:, n * SH:(n + 1) * SH], wk[:, k, :],
                            hsT[:, k, n * SH:(n + 1) * SH],
                            start=(k == 0), stop=(k == KT - 1),
                        )
                _rope(nc, rope_pool, f32, pk, kT[:, g, C:C + S], cs_cc, cs_pm, mult)

            # q projection + rope
            for h in range(NH):
                if h in wqs:
                    wq = wqs[h]
                else:
                    wq = wq_pool.tile([P, KT, P], f32r, tag="wq", name="wq_sb")
                    nc.sync.dma_start(wq[:], wq_d[h].rearrange("k p m -> p k m"))
                pq = ps1.tile([P, S], f32, tag="pqk", bufs=2, name="pq")
                for k in range(KT):
                    for n in range(2):
                        _mm(
    nc,
                            pq[:, n * SH:(n + 1) * SH], wq[:, k, :],
                            hsT[:, k, n * SH:(n + 1) * SH],
                            start=(k == 0), stop=(k == KT - 1),
                        )
                _rope(nc, rope_pool, f32, pq, qT[:, h, :], cs_cc, cs_pm, mult)

            # v projection (natural layout: tokens on partitions)
            for mv in range(S // P):
                pv = ps1.tile([P, NG * P], f32, tag="pv", bufs=2, name="pv")
                for k in range(KT):
                    _mm(
    nc,
                        pv[:], hsT[:, k, mv * P:(mv + 1) * P], wv[:, k, :],
                        start=(k == 0), stop=(k == KT - 1),
                    )
                nc.vector.tensor_copy(vF[:, C // P + mv, :, :], pv[:])

        # ---------------- phase 2: attention + output projection ----------------
        with (
            tc.tile_pool(name="attn_sb", bufs=1) as attn_pool,
            tc.tile_pool(name="probs", bufs=1) as probs_pool,
            tc.tile_pool(name="wo_pool", bufs=3) as wo_pool,
            tc.tile_pool(name="small", bufs=2) as small_pool,
            tc.tile_pool(name="ps2", bufs=1, space="PSUM") as ps2,
        ):
            attn = attn_pool.tile([P, NH, S], f32r, name="attn_sb")

            for sh in range(2):
                ssl = slice(sh * SH, (sh + 1) * SH)
                for h in range(NH):
                    g = h // (NH // NG)
                    NCG = KC // 2  # chunk groups of 2
                    pscores = [None] * NCG
                    probs = [None] * NCG
                    pav = ps2.tile([P, SH], f32, tag="av", bufs=1, name="pav")
                    pden = ps2.tile([P, SH], f32, tag="den", bufs=1, name="pden")

                    def emit_scores(cg):
                        ps = ps2.tile([P, 2, SH], f32, tag="score", bufs=2,
                                      name="pscore")
                        for j in range(2):
                            c = cg * 2 + j
                            _mm(
    nc,
                                ps[:, j, :], kT[:, g, c * P:(c + 1) * P],
                                qT[:, h, ssl], start=True, stop=True,
                            )
                        pt = probs_pool.tile([P, 2, SH], f32r, tag="probs",
                                             bufs=8, name="probs_t")
                        nc.scalar.activation(pt[:], ps[:], Exp)
                        pscores[cg] = ps
                        probs[cg] = pt

                    def emit_avden(cg):
                        for j in range(2):
                            c = cg * 2 + j
                            first, last = (c == 0), (c == KC - 1)
                            _mm(
    nc,
                                pav[:], vF[:, c, g, :], probs[cg][:, j, :],
                                start=first, stop=last, skip_group_check=True,
                            )
                            _mm(
    nc,
                                pden[:], ones128[:], probs[cg][:, j, :],
                                start=first, stop=last, skip_group_check=True,
                            )

                    # software pipeline: scores run 2 groups ahead of AV/den
                    for cg in range(NCG + 2):
                        if cg < NCG:
                            emit_scores(cg)
                        if cg >= 2:
                            emit_avden(cg - 2)

                    # reciprocal of the denominator, broadcast over partitions
                    denr = small_pool.tile([1, SH], f32, tag="denr", name="denr")
                    nc.vector.reciprocal_approx_fast(out=denr[:], in_=pden[0:1, :])
                    denr_r = small_pool.tile([1, SH], f32r, tag="denr_r", name="denr_r")
                    nc.vector.tensor_copy(denr_r[:], denr[:])
                    _mm(nc, pden[:], ones1[:], denr_r[:], start=True, stop=True)
                    rbc = small_pool.tile([P, SH], f32, tag="rbc", name="rbc")
                    nc.vector.tensor_copy(rbc[:], pden[:])
                    # normalized attention output (transposed), fused copyback
                    nc.vector.tensor_tensor(attn[:, h, ssl], pav[:], rbc[:], mult)

                # output projection for this s-half (overlaps next half's attn)
                wons = []
                for n in range(HID // SH):
                    won = wo_pool.tile([P, NH, SH], f32r, tag="won", name="won")
                    nc.sync.dma_start(
                        won[:],
                        wo_d[:, :, n * SH:(n + 1) * SH].rearrange("h p n -> p h n"),
                    )
                    wons.append(won)
                for n in range(HID // SH):
                    won = wons[n]
                    for mt in range(4):
                        m = sh * 4 + mt
                        py = ps2.tile([P, SH], f32, tag="py", bufs=2, name="py")
                        for h in range(NH):
                            _mm(
    nc,
                                py[:], attn[:, h, m * P:(m + 1) * P], won[:, h, :],
                                start=(h == 0), stop=(h == NH - 1),
                            )
                        ysb = small_pool.tile([P, SH], f32, tag="ysb", name="ysb")
                        nc.vector.tensor_copy(ysb[:], py[:])
                        nc.sync.dma_start(
                            y_d[m * P:(m + 1) * P, n * SH:(n + 1) * SH], ysb[:],
                        )


def prep_inputs(hidden_states, freqs_cos, freqs_sin, cache_k, cache_v,
                wq, wk, wv, wo):
    """Shard + pre-transpose the full inputs into 8 per-core input maps."""
    f = np.float32
    scale = np.float32(1.0 / np.sqrt(D))
    wq_p = (wq.astype(f).reshape(H, D, HID)[:, _PERM, :] * scale)
    wk_p = wk.astype(f).reshape(G, D, HID)[:, _PERM, :]
    wv_r = wv.astype(f).reshape(G, D, HID)

    cc = freqs_cos.astype(f).T          # [64, S]
    ss = freqs_sin.astype(f).T
    cs_cc = np.ascontiguousarray(np.concatenate([cc, cc], axis=0))
    cs_pm = np.ascontiguousarray(np.concatenate([ss, -ss], axis=0))

    in_maps = []
    for b in range(B):
        hsT = np.ascontiguousarray(
            hidden_states[b].astype(f).T.reshape(KT, P, S))
        for hg in range(2):
            hs_q = slice(hg * NH, (hg + 1) * NH)
            hs_kv = slice(hg * NG, (hg + 1) * NG)
            wqT = wq_p[hs_q].reshape(NH * D, HID).T          # [HID, 1024]
            wqT_t = np.ascontiguousarray(
                wqT.reshape(KT, P, NH, P).transpose(2, 0, 1, 3))
            wkT = wk_p[hs_kv].reshape(NG * D, HID).T         # [HID, 256]
            wkT_t = np.ascontiguousarray(
                wkT.reshape(KT, P, NG, P).transpose(2, 0, 1, 3))
            wvT = wv_r[hs_kv].reshape(NG * D, HID).T         # [HID, 256]
            wvT_t = np.ascontiguousarray(wvT.reshape(KT, P, NG * P))
            woT = np.ascontiguousarray(
                wo.astype(f)[:, hg * NH * D:(hg + 1) * NH * D].T
                .reshape(NH, P, HID))
            ckT = np.ascontiguousarray(
                cache_k[b].astype(f)[:, hs_kv][:, :, _PERM].transpose(1, 2, 0))
            cvP = np.ascontiguousarray(
                cache_v[b].astype(f)[:, hs_kv]
                .reshape(C // P, P, NG, P).transpose(1, 0, 2, 3))
            in_maps.append({
                "hsT": hsT, "wqT": wqT_t, "wkT": wkT_t, "wvT": wvT_t,
                "woT": woT, "ckT": ckT, "cvP": cvP,
                "cs_cc": cs_cc, "cs_pm": cs_pm,
            })
    return in_maps


def gather_output(results):
    """Sum the 2 TP partials per batch -> full [B, S, HID] output."""
    out = np.empty((B, S, HID), np.float32)
    for b in range(B):
        out[b] = results[2 * b]["y"] + results[2 * b + 1]["y"]
    return out


def kernel(hidden_states, freqs_cos, freqs_sin, attention_mask,
           cache_k, cache_v, wq, wk, wv, wo):
    # attention_mask is all-zeros by construction (see spec) - unused.
    from concourse.bass_utils import run_bass_kernel_spmd

    nc = build_bass(unroll=1)
    in_maps = prep_inputs(
        np.asarray(hidden_states), np.asarray(freqs_cos), np.asarray(freqs_sin),
        np.asarray(cache_k), np.asarray(cache_v),
        np.asarray(wq), np.asarray(wk), np.asarray(wv), np.asarray(wo))
    res = run_bass_kernel_spmd(nc, in_maps, core_ids=list(range(N_CORES)))
    return gather_output(res.results)



# revision 9
# speedup vs baseline: 1.3363x; 1.3363x over previous
"""BrahmaAttention (GQA prefill with KV cache) on 8 Trainium2 NeuronCores.

Problem: B=4, S=1024, C=1024 (cache), H=16 q-heads, G=4 kv-heads, D=128.
    q = hs @ wq.T ; k = hs @ wk.T ; v = hs @ wv.T
    rope(q, k) (interleaved pairs, positions C..C+S)
    k_full/v_full = concat(cache, new)           # K = 2048 keys
    out = softmax(q k^T / sqrt(D)) @ v_full @ wo.T
(attention_mask is all-zeros by construction - full attention, no masking.)

Sharding: 4-way data parallel over batch x 2-way tensor parallel over heads.
core (b, hg) handles batch b, q-heads hg*8..hg*8+8, kv-heads hg*2..hg*2+2 and
computes a partial output projection over its 1024 hidden columns; the host
sums the two partials per batch (the TP all-reduce done on host at gather).

Host-side prep folded into the shards:
  - 1/sqrt(D) folded into wq.
  - RoPE even/odd interleave permuted to [evens|odds] via wq/wk row
    permutation and cache_k last-dim permutation, so on-chip RoPE is
    half-tile elementwise ops (partitions 0-63 = even, 64-127 = odd lanes).
  - All projection weights pre-transposed/tiled so every DMA is contiguous
    and every matmul operand is naturally [K=128, *] in SBUF.

On-chip layout per core: everything transposed, [D, S]-style:
  qT [128, 8, 1024]  kT_full [128, 2, 2048]  v_full [128, 16, 2, 128]
  scoresT = kT.T @ qT per head -> [keys, s] in PSUM -> exp on ACT -> probs
  denominator: ones[128,128].T @ probs accumulated on PE, reciprocal via
  DVE approx, broadcast back over partitions with a K=1 matmul, and the
  normalization is fused into the AV PSUM->SBUF copyback.
"""

import numpy as np

B, S, C, H, G, D = 4, 1024, 1024, 16, 4, 128
HID = H * D
P = 128
NH, NG = 8, 2          # per-core q heads / kv heads
KC = (C + S) // P      # 16 key chunks
KT = 16                # hid contraction tiles
SH = 512               # s-half (PSUM bank free size)
N_CORES = 8

_PERM = np.concatenate([np.arange(0, D, 2), np.arange(1, D, 2)])

_BUILT = {}
_MM_F32R = True


def _mm(nc, out, lhsT, rhs, **kw):
    """All matmul operands live in float32r tiles (TF32-like: fp32 storage,
    rounded mantissa, 4x the PE streaming rate of full fp32)."""
    nc.tensor.matmul(out, lhsT, rhs, **kw)



def _rope(nc, pool, f32, psum_in, out_ap, cs_cc, cs_pm, mult):
    """out = psum_in*[cos;cos] + swap_halves(psum_in*[sin;-sin]).

    psum_in is the raw projected [128, S] tile with evens on partitions 0-63
    and odds on 64-127; out gets the roped value in the same layout.
    """
    import concourse.mybir as mybir

    a = pool.tile([P, S], f32, tag="ropeA", name="ropeA")
    b = pool.tile([P, S], f32, tag="ropeB", name="ropeB")
    s = pool.tile([P, S], f32, tag="ropeS", name="ropeS")
    nc.vector.tensor_tensor(a[:], psum_in[:], cs_cc[:], mult)
    nc.vector.tensor_tensor(b[:], psum_in[:], cs_pm[:], mult)
    # swap halves on the scalar engine (idle during phase 1)
    nc.scalar.copy(s[0:64, :], b[64:128, :])
    nc.scalar.copy(s[64:128, :], b[0:64, :])
    nc.vector.tensor_tensor(out_ap, a[:], s[:], mybir.AluOpType.add)


def build_bass(unroll=1):
    """Build + compile the per-core Bass program (identical on all cores)."""
    if unroll in _BUILT:
        return _BUILT[unroll]

    import concourse.mybir as mybir
    import concourse.tile as tile
    from concourse import bacc

    f32 = mybir.dt.float32
    f32r = mybir.dt.float32r if _MM_F32R else mybir.dt.float32
    bf16 = mybir.dt.bfloat16
    mult = mybir.AluOpType.mult
    add = mybir.AluOpType.add
    Exp = mybir.ActivationFunctionType.Exp

    nc = bacc.Bacc("TRN2", target_bir_lowering=False, debug=False)

    hsT_d = nc.dram_tensor("hsT", [KT, P, S], f32r, kind="ExternalInput")
    wq_d = nc.dram_tensor("wqT", [NH, KT, P, P], f32r, kind="ExternalInput")
    wk_d = nc.dram_tensor("wkT", [NG, KT, P, P], f32r, kind="ExternalInput")
    wv_d = nc.dram_tensor("wvT", [KT, P, NG * P], f32r, kind="ExternalInput")
    wo_d = nc.dram_tensor("woT", [NH, P, HID], f32r, kind="ExternalInput")
    ck_d = nc.dram_tensor("ckT", [NG, P, C], f32r, kind="ExternalInput")
    cv_d = nc.dram_tensor("cvP", [P, C // P, NG, P], mybir.dt.bfloat16,
                          kind="ExternalInput")
    cc_d = nc.dram_tensor("cs_cc", [P, S], f32, kind="ExternalInput")
    pm_d = nc.dram_tensor("cs_pm", [P, S], f32, kind="ExternalInput")
    y_d = nc.dram_tensor("y", [S, HID], f32, kind="ExternalOutput")

    with tile.TileContext(nc) as tc:
        with tc.tile_pool(name="const", bufs=1) as const:
            ones_f = const.tile([P, P], f32, name="ones_f")
            nc.any.memset(ones_f[:], 1.0)
            ones128 = const.tile([P, P], f32r, name="ones128")
            nc.vector.tensor_copy(ones128[:], ones_f[:])
            ones_bf = const.tile([P, P], bf16, name="ones_bf")
            nc.vector.tensor_copy(ones_bf[:], ones_f[:])
            cs_cc = const.tile([P, S], f32, name="cs_cc")
            cs_pm = const.tile([P, S], f32, name="cs_pm")

            for it in range(unroll):
                _emit_iteration(
                    nc, tc, f32, f32r, bf16, mult, add, Exp,
                    hsT_d, wq_d, wk_d, wv_d, wo_d, ck_d, cv_d, y_d,
                    ones128, ones_bf, cs_cc, cs_pm,
                    cs_load=(cc_d, pm_d) if it == 0 else None,
                )

    nc.compile()
    _BUILT[unroll] = nc
    return nc


def _emit_iteration(nc, tc, f32, f32r, bf16, mult, add, Exp,
                    hsT_d, wq_d, wk_d, wv_d, wo_d, ck_d, cv_d, y_d,
                    ones128, ones_bf, cs_cc, cs_pm, cs_load=None):
    import concourse.tile as tile  # noqa: F401

    with tc.tile_pool(name="persist", bufs=1) as persist:
        qT = persist.tile([P, NH, S], f32r, name="qT")
        kT = persist.tile([P, NG, C + S], f32r, name="kT")
        vF = persist.tile([P, KC, NG, P], bf16, name="vF")

        # ---------------- phase 1: projections + rope ----------------
        with (
            tc.tile_pool(name="ph1", bufs=1) as ph1,
            tc.tile_pool(name="wq_pool", bufs=2) as wq_pool,
            tc.tile_pool(name="wk_pool", bufs=2) as wk_pool,
            tc.tile_pool(name="rope", bufs=1) as rope_pool,
            tc.tile_pool(name="ps1", bufs=1, space="PSUM") as ps1,
        ):
            if cs_load is not None:
                # PE/HAM warm-up during the initial DMA window (iteration 0
                # only): ~130 back-to-back tiny matmuls, consumed by one copy
                # so they survive DCE.
                pw = ps1.tile([P, P], f32, tag="warm", bufs=1, name="pwarm")
                for i in range(150):
                    _mm(nc, pw[:], ones128[:], ones128[:],
                        start=(i == 0), stop=(i == 149), skip_group_check=True)
                wsink = rope_pool.tile([1, 1], f32, tag="wsink", name="wsink")
                nc.vector.tensor_copy(wsink[:], pw[0:1, 0:1])
            # DMA emission order = queue order: first matmul needs wk half 0
            # + hsT chunk 0, so those go first; bulk loads (cache, wv) that
            # are only needed later in the phase go last.
            wks = []
            hsT = ph1.tile([P, KT, S], f32r, name="hsT_sb")
            for g in range(NG):
                wk = wk_pool.tile([P, KT, P], f32r, tag="wk", name="wk_sb")
                nc.sync.dma_start(wk[:, 0:KT // 2, :],
                                  wk_d[g, 0:KT // 2].rearrange("k p m -> p k m"))
                wks.append(wk)
                if g == 0:
                    nc.sync.dma_start(
                        hsT[:, 0:2, :], hsT_d[0:2].rearrange("k p s -> p k s"))
            for g in range(NG):
                nc.sync.dma_start(wks[g][:, KT // 2:, :],
                                  wk_d[g, KT // 2:].rearrange("k p m -> p k m"))
            if cs_load is not None:
                nc.sync.dma_start(cs_cc[:], cs_load[0][:])
                nc.sync.dma_start(cs_pm[:], cs_load[1][:])
            for i in range(1, 8):
                nc.sync.dma_start(
                    hsT[:, 2 * i:2 * i + 2, :],
                    hsT_d[2 * i:2 * i + 2].rearrange("k p s -> p k s"),
                )
            # q weights for the first heads ahead of the bulk cache loads
            wqs = {}
            for h in range(2):
                wq = wq_pool.tile([P, KT, P], f32r, tag="wq", name="wq_sb")
                nc.sync.dma_start(wq[:], wq_d[h].rearrange("k p m -> p k m"))
                wqs[h] = wq
            for g in range(NG):
                nc.sync.dma_start(kT[:, g, 0:C], ck_d[g])
            nc.sync.dma_start(vF[:, 0:C // P, :, :], cv_d[:])
            wv = ph1.tile([P, KT, NG * P], f32r, name="wv_sb")
            nc.sync.dma_start(wv[:], wv_d.rearrange("k p n -> p k n"))

            # k projection + rope (new keys go to kT[:, g, C:])
            for g in range(NG):
                wk = wks[g]
                pk = ps1.tile([P, S], f32, tag="pqk", bufs=2, name="pk")
                for k in range(KT):
                    for n in range(2):
                        _mm(
    nc,
                            pk[:, n * SH:(n + 1) * SH], wk[:, k, :],
                            hsT[:, k, n * SH:(n + 1) * SH],
                            start=(k == 0), stop=(k == KT - 1),
                        )
                _rope(nc, rope_pool, f32, pk, kT[:, g, C:C + S], cs_cc, cs_pm, mult)

            # q projection + rope
            for h in range(NH):
                if h in wqs:
                    wq = wqs[h]
                else:
                    wq = wq_pool.tile([P, KT, P], f32r, tag="wq", name="wq_sb")
                    nc.sync.dma_start(wq[:], wq_d[h].rearrange("k p m -> p k m"))
                pq = ps1.tile([P, S], f32, tag="pqk", bufs=2, name="pq")
                for k in range(KT):
                    for n in range(2):
                        _mm(
    nc,
                            pq[:, n * SH:(n + 1) * SH], wq[:, k, :],
                            hsT[:, k, n * SH:(n + 1) * SH],
                            start=(k == 0), stop=(k == KT - 1),
                        )
                _rope(nc, rope_pool, f32, pq, qT[:, h, :], cs_cc, cs_pm, mult)

            # v projection (natural layout: tokens on partitions)
            for mv in range(S // P):
                pv = ps1.tile([P, NG * P], f32, tag="pv", bufs=2, name="pv")
                for k in range(KT):
                    _mm(
    nc,
                        pv[:], hsT[:, k, mv * P:(mv + 1) * P], wv[:, k, :],
                        start=(k == 0), stop=(k == KT - 1),
                    )
                nc.vector.tensor_copy(vF[:, C // P + mv, :, :], pv[:])

        # ---------------- phase 2: attention + output projection ----------------
        with (
            tc.tile_pool(name="attn_sb", bufs=1) as attn_pool,
            tc.tile_pool(name="probs", bufs=1) as probs_pool,
            tc.tile_pool(name="wo_pool", bufs=3) as wo_pool,
            tc.tile_pool(name="small", bufs=2) as small_pool,
            tc.tile_pool(name="ps2", bufs=1, space="PSUM") as ps2,
        ):
            attn = attn_pool.tile([P, NH, S], f32r, name="attn_sb")

            for sh in range(2):
                ssl = slice(sh * SH, (sh + 1) * SH)
                for h in range(NH):
                    g = h // (NH // NG)
                    NCG = KC // 2  # chunk groups of 2
                    pscores = [None] * NCG
                    probs = [None] * NCG
                    pav = ps2.tile([P, SH], f32, tag="av", bufs=1, name="pav")
                    # bf16 partial-sum accumulators for the softmax denominator
                    # (DVE add-tree over the probs tiles; the cross-partition
                    # sum is ONE ones-matmul whose output is den broadcast to
                    # all partitions for free).
                    accA = small_pool.tile([P, 2, SH], bf16, tag="accA",
                                           name="accA")
                    accB = small_pool.tile([P, 2, SH], bf16, tag="accB",
                                           name="accB")

                    def emit_scores(cg):
                        ps = ps2.tile([P, 2, SH], f32, tag="score", bufs=2,
                                      name="pscore")
                        for j in range(2):
                            c = cg * 2 + j
                            _mm(
    nc,
                                ps[:, j, :], kT[:, g, c * P:(c + 1) * P],
                                qT[:, h, ssl], start=True, stop=True,
                            )
                        pt = probs_pool.tile([P, 2, SH], bf16, tag="probs",
                                             bufs=8, name="probs_t")
                        nc.scalar.activation(pt[:], ps[:], Exp)
                        pscores[cg] = ps
                        probs[cg] = pt

                    def emit_av(cg):
                        for j in range(2):
                            c = cg * 2 + j
                            first, last = (c == 0), (c == KC - 1)
                            _mm(
    nc,
                                pav[:], vF[:, c, g, :], probs[cg][:, j, :],
                                start=first, stop=last, skip_group_check=True,
                            )

                    # software pipeline: scores run 2 groups ahead of AV;
                    # denominator partial sums accumulate on DVE as the probs
                    # tiles land (two independent chains to halve the latency).
                    for cg in range(NCG + 2):
                        if cg < NCG:
                            emit_scores(cg)
                        if cg == 2:
                            nc.vector.tensor_tensor(
                                accA[:], probs[0][:], probs[2][:], add)
                        elif cg == 3:
                            nc.vector.tensor_tensor(
                                accB[:], probs[1][:], probs[3][:], add)
                        elif cg in (4, 5, 6, 7):
                            acc = accA if cg % 2 == 0 else accB
                            nc.vector.tensor_tensor(
                                acc[:], acc[:], probs[cg][:], add)
                        if cg >= 2:
                            emit_av(cg - 2)

                    # fold the two chains and the chunk pairs -> [P, SH]
                    nc.vector.tensor_tensor(accA[:], accA[:], accB[:], add)
                    denF = small_pool.tile([P, SH], bf16, tag="denF",
                                           name="denF")
                    nc.vector.tensor_tensor(
                        denF[:], accA[:, 0, :], accA[:, 1, :], add)
                    # cross-partition sum; output = den on ALL 128 partitions
                    pden = ps2.tile([P, SH], f32, tag="den", bufs=1,
                                    name="pden")
                    _mm(nc, pden[:], ones_bf[:], denF[:], start=True, stop=True)
                    # early PSUM->SBUF copy frees the AV bank for the next head
                    araw = small_pool.tile([P, SH], f32r, tag="araw",
                                           name="araw")
                    nc.vector.tensor_copy(araw[:], pav[:])
                    rcp = small_pool.tile([P, SH], f32, tag="rcp", name="rcp")
                    nc.vector.reciprocal_approx_fast(out=rcp[:], in_=pden[:])
                    # normalized attention output (transposed)
                    nc.vector.tensor_tensor(attn[:, h, ssl], araw[:], rcp[:],
                                            mult)

                # output projection for this s-half (overlaps next half's attn)
                wons = []
                for n in range(HID // SH):
                    won = wo_pool.tile([P, NH, SH], f32r, tag="won", name="won")
                    nc.sync.dma_start(
                        won[:],
                        wo_d[:, :, n * SH:(n + 1) * SH].rearrange("h p n -> p h n"),
                    )
                    wons.append(won)
                for n in range(HID // SH):
                    won = wons[n]
                    for mt in range(4):
                        m = sh * 4 + mt
                        py = ps2.tile([P, SH], f32, tag="py", bufs=2, name="py")
                        for h in range(NH):
                            _mm(
    nc,
                                py[:], attn[:, h, m * P:(m + 1) * P], won[:, h, :],
                                start=(h == 0), stop=(h == NH - 1),
                            )
                        ysb = small_pool.tile([P, SH], f32, tag="ysb", name="ysb")
                        nc.vector.tensor_copy(ysb[:], py[:])
                        nc.sync.dma_start(
                            y_d[m * P:(m + 1) * P, n * SH:(n + 1) * SH], ysb[:],
                        )


def prep_inputs(hidden_states, freqs_cos, freqs_sin, cache_k, cache_v,
                wq, wk, wv, wo):
    """Shard + pre-transpose the full inputs into 8 per-core input maps."""
    import ml_dtypes
    bf = ml_dtypes.bfloat16
    f = np.float32
    scale = np.float32(1.0 / np.sqrt(D))
    wq_p = (wq.astype(f).reshape(H, D, HID)[:, _PERM, :] * scale)
    wk_p = wk.astype(f).reshape(G, D, HID)[:, _PERM, :]
    wv_r = wv.astype(f).reshape(G, D, HID)

    cc = freqs_cos.astype(f).T          # [64, S]
    ss = freqs_sin.astype(f).T
    cs_cc = np.ascontiguousarray(np.concatenate([cc, cc], axis=0))
    cs_pm = np.ascontiguousarray(np.concatenate([ss, -ss], axis=0))

    in_maps = []
    for b in range(B):
        hsT = np.ascontiguousarray(
            hidden_states[b].astype(f).T.reshape(KT, P, S))
        for hg in range(2):
            hs_q = slice(hg * NH, (hg + 1) * NH)
            hs_kv = slice(hg * NG, (hg + 1) * NG)
            wqT = wq_p[hs_q].reshape(NH * D, HID).T          # [HID, 1024]
            wqT_t = np.ascontiguousarray(
                wqT.reshape(KT, P, NH, P).transpose(2, 0, 1, 3))
            wkT = wk_p[hs_kv].reshape(NG * D, HID).T         # [HID, 256]
            wkT_t = np.ascontiguousarray(
                wkT.reshape(KT, P, NG, P).transpose(2, 0, 1, 3))
            wvT = wv_r[hs_kv].reshape(NG * D, HID).T         # [HID, 256]
            wvT_t = np.ascontiguousarray(wvT.reshape(KT, P, NG * P))
            woT = np.ascontiguousarray(
                wo.astype(f)[:, hg * NH * D:(hg + 1) * NH * D].T
                .reshape(NH, P, HID))
            ckT = np.ascontiguousarray(
                cache_k[b].astype(f)[:, hs_kv][:, :, _PERM].transpose(1, 2, 0))
            cvP = np.ascontiguousarray(
                cache_v[b].astype(f)[:, hs_kv]
                .reshape(C // P, P, NG, P).transpose(1, 0, 2, 3)).astype(bf)
            in_maps.append({
                "hsT": hsT, "wqT": wqT_t, "wkT": wkT_t, "wvT": wvT_t,
                "woT": woT, "ckT": ckT, "cvP": cvP,
                "cs_cc": cs_cc, "cs_pm": cs_pm,
            })
    return in_maps


def gather_output(results):
    """Sum the 2 TP partials per batch -> full [B, S, HID] output."""
    out = np.empty((B, S, HID), np.float32)
    for b in range(B):
        out[b] = results[2 * b]["y"] + results[2 * b + 1]["y"]
    return out


def kernel(hidden_states, freqs_cos, freqs_sin, attention_mask,
           cache_k, cache_v, wq, wk, wv, wo):
    # attention_mask is all-zeros by construction (see spec) - unused.
    from concourse.bass_utils import run_bass_kernel_spmd

    nc = build_bass(unroll=1)
    in_maps = prep_inputs(
        np.asarray(hidden_states), np.asarray(freqs_cos), np.asarray(freqs_sin),
        np.asarray(cache_k), np.asarray(cache_v),
        np.asarray(wq), np.asarray(wk), np.asarray(wv), np.asarray(wo))
    res = run_bass_kernel_spmd(nc, in_maps, core_ids=list(range(N_CORES)))
    return gather_output(res.results)



# revision 14
# speedup vs baseline: 3.0613x; 2.2908x over previous
"""BrahmaAttention (GQA prefill with KV cache) on 8 Trainium2 NeuronCores.

Problem: B=4, S=1024, C=1024 (cache), H=16 q-heads, G=4 kv-heads, D=128.
    q = hs @ wq.T ; k = hs @ wk.T ; v = hs @ wv.T
    rope(q, k) (interleaved pairs, positions C..C+S)
    k_full/v_full = concat(cache, new)           # K = 2048 keys
    out = softmax(q k^T / sqrt(D)) @ v_full @ wo.T
(attention_mask is all-zeros by construction - full attention, no masking.)

Sharding: 4-way data parallel over batch x 2-way tensor parallel over heads.
core (b, hg) handles batch b, q-heads hg*8..hg*8+8, kv-heads hg*2..hg*2+2 and
computes a partial output projection over its 1024 hidden columns; the host
sums the two partials per batch (the TP all-reduce done on host at gather).

Host-side prep folded into the shards:
  - 1/sqrt(D) folded into wq.
  - RoPE even/odd interleave permuted to [evens|odds] via wq/wk row
    permutation and cache_k last-dim permutation, so on-chip RoPE is
    half-tile elementwise ops (partitions 0-63 = even, 64-127 = odd lanes).
  - All projection weights pre-transposed/tiled (bf16) so every DMA is
    contiguous and every matmul operand is naturally [K=128, *] in SBUF.

Engine balance (the point of this structure): the PE streams 1 row/cycle
for both f32r and bf16, so all matmuls are minimal-row already; exp runs
only on ACT (~8.6us per head-block, just above a head-block's 7us of PE
work), so attention alone is ACT-bound while the projections are PE-bound
with ACT idle.  The kernel therefore interleaves:
  - Q-projections for heads 2..7 into the first attention half,
  - the first half's output projection into the second attention half,
so the PE always has projection work to fill ACT-gated gaps.  The softmax
denominator never touches the PE: bf16 probs tiles accumulate on the DVE
(two chains), are cross-partition-reduced on the idle Pool engine
(partition_all_reduce), inverted with the fast DVE reciprocal, and the
normalization is fused into the PSUM->SBUF copy of the AV output.
"""

import numpy as np

B, S, C, H, G, D = 4, 1024, 1024, 16, 4, 128
HID = H * D
P = 128
NH, NG = 8, 2          # per-core q heads / kv heads
KC = (C + S) // P      # 16 key chunks
KT = 16                # hid contraction tiles
SH = 512               # s-half (PSUM bank free size)
N_CORES = 8

_PERM = np.concatenate([np.arange(0, D, 2), np.arange(1, D, 2)])

_BUILT = {}


def _mm(nc, out, lhsT, rhs, **kw):
    nc.tensor.matmul(out, lhsT, rhs, **kw)


def _rope_half(nc, pool, f32, psum_in, out_ap, cs_cc, cs_pm, mult, add):
    """out = psum_in*[cos;cos] + swap_partition_halves(psum_in*[sin;-sin]).

    psum_in is a raw projected [128, SH] tile with evens on partitions 0-63
    and odds on 64-127; out gets the roped value in the same layout.
    """
    a = pool.tile([P, SH], f32, tag="ropeA", name="ropeA")
    b = pool.tile([P, SH], f32, tag="ropeB", name="ropeB")
    s = pool.tile([P, SH], f32, tag="ropeS", name="ropeS")
    nc.vector.tensor_tensor(a[:], psum_in[:], cs_cc[:], mult)
    nc.vector.tensor_tensor(b[:], psum_in[:], cs_pm[:], mult)
    # swap partition halves on the scalar engine
    nc.scalar.copy(s[0:64, :], b[64:128, :])
    nc.scalar.copy(s[64:128, :], b[0:64, :])
    nc.vector.tensor_tensor(out_ap, a[:], s[:], add)


def build_bass(unroll=1):
    """Build + compile the per-core Bass program (identical on all cores)."""
    if unroll in _BUILT:
        return _BUILT[unroll]

    import concourse.mybir as mybir
    import concourse.tile as tile
    from concourse import bacc

    f32 = mybir.dt.float32
    f32r = mybir.dt.float32r
    bf16 = mybir.dt.bfloat16
    mult = mybir.AluOpType.mult
    add = mybir.AluOpType.add
    Exp = mybir.ActivationFunctionType.Exp

    nc = bacc.Bacc("TRN2", target_bir_lowering=False, debug=False)

    hsT_d = nc.dram_tensor("hsT", [KT, P, S], bf16, kind="ExternalInput")
    wq_d = nc.dram_tensor("wqT", [NH, KT, P, P], bf16, kind="ExternalInput")
    wk_d = nc.dram_tensor("wkT", [NG, KT, P, P], bf16, kind="ExternalInput")
    wv_d = nc.dram_tensor("wvT", [KT, P, NG * P], bf16, kind="ExternalInput")
    wo_d = nc.dram_tensor("woT", [NH, P, HID], bf16, kind="ExternalInput")
    ck_d = nc.dram_tensor("ckT", [NG, P, C], f32r, kind="ExternalInput")
    cv_d = nc.dram_tensor("cvP", [P, C // P, NG * P], bf16,
                          kind="ExternalInput")
    cc_d = nc.dram_tensor("cs_cc", [P, S], f32, kind="ExternalInput")
    pm_d = nc.dram_tensor("cs_pm", [P, S], f32, kind="ExternalInput")
    y_d = nc.dram_tensor("y", [S, HID], f32, kind="ExternalOutput")

    with tile.TileContext(nc) as tc:
        with tc.tile_pool(name="const", bufs=1) as const:
            ones_f = const.tile([P, P], f32, name="ones_f")
            nc.any.memset(ones_f[:], 1.0)
            ones128 = const.tile([P, P], f32r, name="ones128")
            nc.vector.tensor_copy(ones128[:], ones_f[:])
            cs_cc = const.tile([P, S], f32, name="cs_cc")
            cs_pm = const.tile([P, S], f32, name="cs_pm")

            for it in range(unroll):
                _emit_iteration(
                    nc, tc, f32, f32r, bf16, mult, add, Exp,
                    hsT_d, wq_d, wk_d, wv_d, wo_d, ck_d, cv_d, y_d,
                    ones128, cs_cc, cs_pm,
                    cs_load=(cc_d, pm_d) if it == 0 else None,
                )

    nc.compile()
    _BUILT[unroll] = nc
    return nc


def _emit_iteration(nc, tc, f32, f32r, bf16, mult, add, Exp,
                    hsT_d, wq_d, wk_d, wv_d, wo_d, ck_d, cv_d, y_d,
                    ones128, cs_cc, cs_pm, cs_load=None):
    from concourse import bass_isa

    with (
        tc.tile_pool(name="persist", bufs=1) as persist,
        tc.tile_pool(name="proj", bufs=1) as proj_pool,
        tc.tile_pool(name="wq_pool", bufs=2) as wq_pool,
        tc.tile_pool(name="wk_pool", bufs=2) as wk_pool,
        tc.tile_pool(name="rope", bufs=1) as rope_pool,
        tc.tile_pool(name="attn_sb", bufs=1) as attn_pool,
        tc.tile_pool(name="probs", bufs=1) as probs_pool,
        tc.tile_pool(name="wo_pool", bufs=4) as wo_pool,
        tc.tile_pool(name="small", bufs=2) as small_pool,
        tc.tile_pool(name="ps", bufs=1, space="PSUM") as ps,
    ):
        qT = persist.tile([P, NH, S], f32r, name="qT")
        kT = persist.tile([P, NG, C + S], f32r, name="kT")
        vF = persist.tile([P, KC, NG * P], bf16, name="vF")
        attn = attn_pool.tile([P, NH, S], bf16, name="attn_sb")
        hsT = proj_pool.tile([P, KT, S], bf16, name="hsT_sb")
        wv = proj_pool.tile([P, KT, NG * P], bf16, name="wv_sb")

        # ---------------- DMA queue (ordered by first use) ----------------
        if cs_load is not None:
            # PE/HAM warm-up during the initial DMA window (iteration 0
            # only): ~150 back-to-back tiny matmuls, consumed by one copy
            # so they survive DCE.
            pw = ps.tile([P, SH], f32, tag="av", bufs=2, name="pwarm")
            for i in range(150):
                _mm(nc, pw[:, 0:P], ones128[:], ones128[:],
                    start=(i == 0), stop=(i == 149), skip_group_check=True)
            wsink = small_pool.tile([1, 1], f32, tag="wsink", name="wsink")
            nc.vector.tensor_copy(wsink[:], pw[0:1, 0:1])
            nc.sync.dma_start(cs_cc[:], cs_load[0][:])
            nc.sync.dma_start(cs_pm[:], cs_load[1][:])
        wks = []
        for g in range(NG):
            wk = wk_pool.tile([P, KT, P], bf16, tag="wk", name="wk_sb")
            nc.sync.dma_start(wk[:], wk_d[g].rearrange("k p m -> p k m"))
            wks.append(wk)
            if g == 0:
                for i in range(8):
                    nc.sync.dma_start(
                        hsT[:, 2 * i:2 * i + 2, :],
                        hsT_d[2 * i:2 * i + 2].rearrange("k p s -> p k s"),
                    )
        wqs = {}
        for h in range(2):
            wq = wq_pool.tile([P, KT, P], bf16, tag="wq", name="wq_sb")
            nc.sync.dma_start(wq[:], wq_d[h].rearrange("k p m -> p k m"))
            wqs[h] = wq
        nc.sync.dma_start(wv[:], wv_d.rearrange("k p n -> p k n"))
        for g in range(NG):
            nc.sync.dma_start(kT[:, g, 0:C], ck_d[g])
        nc.sync.dma_start(vF[:, 0:C // P, :], cv_d[:])

        # ---------------- projection helper (one [P, SH] half at a time) ---
        def emit_proj(w, dst_fn):
            for half in range(2):
                hsl = slice(half * SH, (half + 1) * SH)
                pp = ps.tile([P, SH], f32, tag="psA", bufs=2, name="pp")
                for k in range(KT):
                    _mm(nc, pp[:], w[:, k, :], hsT[:, k, hsl],
                        start=(k == 0), stop=(k == KT - 1))
                _rope_half(nc, rope_pool, f32, pp, dst_fn(hsl),
                           cs_cc[:, hsl], cs_pm[:, hsl], mult, add)

        # ---- section 1: K-projection + rope (new keys -> kT[:, g, C:]) ----
        for g in range(NG):
            emit_proj(wks[g],
                      lambda hsl, g=g: kT[:, g, C + hsl.start:C + hsl.stop])
        # ---- section 2: Q-projection heads 0, 1 ----
        for h in range(2):
            emit_proj(wqs[h], lambda hsl, h=h: qT[:, h, hsl])
        # ---- section 3: V-projection (tokens on partitions) ----
        for mv in range(S // P // 2):
            pv = ps.tile([P, SH], f32, tag="psA", bufs=2, name="pv")
            for m2 in range(2):
                m = 2 * mv + m2
                vsl = slice(m2 * NG * P, (m2 + 1) * NG * P)
                for k in range(KT):
                    _mm(nc, pv[:, vsl], hsT[:, k, m * P:(m + 1) * P],
                        wv[:, k, :], start=(k == 0), stop=(k == KT - 1))
            nc.vector.tensor_copy(
                vF[:, C // P + 2 * mv:C // P + 2 * mv + 2, :], pv[:])

        # ---------------- attention head-block ----------------
        def attn_head(sh, h):
            ssl = slice(sh * SH, (sh + 1) * SH)
            g = h // (NH // NG)
            NCG = KC // 2  # chunk groups of 2
            probs = [None] * NCG
            pav = ps.tile([P, SH], f32, tag="av", bufs=2, name="pav")
            accA = small_pool.tile([P, SH], bf16, tag="accA", name="accA")
            accB = small_pool.tile([P, SH], bf16, tag="accB", name="accB")

            def emit_scores(cg):
                pssc = ps.tile([P, 2, SH], f32, tag="score", bufs=2,
                               name="pscore")
                for j in range(2):
                    c = cg * 2 + j
                    _mm(nc, pssc[:, j, :], kT[:, g, c * P:(c + 1) * P],
                        qT[:, h, ssl], start=True, stop=True)
                pt = probs_pool.tile([P, 2, SH], bf16, tag="probs",
                                     bufs=6, name="probs_t")
                nc.scalar.activation(pt[:], pssc[:], Exp)
                probs[cg] = pt

            def emit_av(cg):
                for j in range(2):
                    c = cg * 2 + j
                    _mm(nc, pav[:], vF[:, c, g * P:(g + 1) * P],
                        probs[cg][:, j, :],
                        start=(c == 0), stop=(c == KC - 1),
                        skip_group_check=True)

            # software pipeline: scores run 2 groups ahead of AV; the
            # denominator partial sums accumulate on DVE as tiles land
            # (chain A = sub-chunk 0, chain B = sub-chunk 1).
            for cg in range(NCG + 2):
                if cg < NCG:
                    emit_scores(cg)
                if cg == 1:
                    nc.vector.tensor_tensor(
                        accA[:], probs[0][:, 0, :], probs[1][:, 0, :], add)
                    nc.vector.tensor_tensor(
                        accB[:], probs[0][:, 1, :], probs[1][:, 1, :], add)
                elif 2 <= cg < NCG:
                    nc.vector.tensor_tensor(
                        accA[:], accA[:], probs[cg][:, 0, :], add)
                    nc.vector.tensor_tensor(
                        accB[:], accB[:], probs[cg][:, 1, :], add)
                if cg >= 2:
                    emit_av(cg - 2)

            # denominator: fold chains, cross-partition sum on the (idle)
            # Pool engine -- result lands broadcast on all 128 partitions.
            accC = small_pool.tile([P, SH], bf16, tag="accC", name="accC")
            nc.vector.tensor_tensor(accC[:], accA[:], accB[:], add)
            red = small_pool.tile([P, SH], f32, tag="red", name="red")
            nc.gpsimd.partition_all_reduce(
                red[:], accC[:], channels=P, reduce_op=bass_isa.ReduceOp.add)
            rcp = small_pool.tile([P, SH], f32, tag="rcp", name="rcp")
            nc.vector.reciprocal_approx_fast(out=rcp[:], in_=red[:])
            # normalized attention output (transposed), fused copyback
            nc.vector.tensor_tensor(attn[:, h, ssl], pav[:], rcp[:], mult)

        # ---------------- output-projection group ----------------
        def wo_group(sh, gi, wons):
            n, mt = gi // 4, gi % 4
            m = sh * 4 + mt
            py = ps.tile([P, SH], f32, tag="psA", bufs=2, name="py")
            for h in range(NH):
                _mm(nc, py[:], attn[:, h, m * P:(m + 1) * P],
                    wons[n][:, h, :], start=(h == 0), stop=(h == NH - 1))
            ysb = small_pool.tile([P, SH], f32, tag="ysb", name="ysb")
            nc.vector.tensor_copy(ysb[:], py[:])
            nc.sync.dma_start(
                y_d[m * P:(m + 1) * P, n * SH:(n + 1) * SH], ysb[:])

        # ---- section 4: sh0 attention, Q-proj h2..7 interleaved;
        #      wo weights (shared by both halves) prefetch during it ----
        wons = []
        for n in range(HID // SH):
            won = wo_pool.tile([P, NH, SH], bf16, tag="won", name="won")
            nc.sync.dma_start(
                won[:],
                wo_d[:, :, n * SH:(n + 1) * SH].rearrange("h p n -> p h n"),
            )
            wons.append(won)
        for h in range(NH):
            attn_head(0, h)
            if h < NH - 2:
                hq = h + 2
                wq = wq_pool.tile([P, KT, P], bf16, tag="wq", name="wq_sb")
                nc.sync.dma_start(wq[:], wq_d[hq].rearrange("k p m -> p k m"))
                emit_proj(wq, lambda hsl, hq=hq: qT[:, hq, hsl])

        # ---- section 5: sh1 attention, wo(sh0) groups interleaved ----
        for h in range(NH):
            attn_head(1, h)
            wo_group(0, 2 * h, wons)
            wo_group(0, 2 * h + 1, wons)

        # ---- section 6: wo(sh1) ----
        for gi in range(16):
            wo_group(1, gi, wons)


def prep_inputs(hidden_states, freqs_cos, freqs_sin, cache_k, cache_v,
                wq, wk, wv, wo):
    """Shard + pre-transpose the full inputs into 8 per-core input maps."""
    import ml_dtypes
    bf = ml_dtypes.bfloat16
    f = np.float32
    scale = np.float32(1.0 / np.sqrt(D))
    wq_p = (wq.astype(f).reshape(H, D, HID)[:, _PERM, :] * scale)
    wk_p = wk.astype(f).reshape(G, D, HID)[:, _PERM, :]
    wv_r = wv.astype(f).reshape(G, D, HID)

    cc = freqs_cos.astype(f).T          # [64, S]
    ss = freqs_sin.astype(f).T
    cs_cc = np.ascontiguousarray(np.concatenate([cc, cc], axis=0))
    cs_pm = np.ascontiguousarray(np.concatenate([ss, -ss], axis=0))

    in_maps = []
    for b in range(B):
        hsT = np.ascontiguousarray(
            hidden_states[b].astype(f).T.reshape(KT, P, S)).astype(bf)
        for hg in range(2):
            hs_q = slice(hg * NH, (hg + 1) * NH)
            hs_kv = slice(hg * NG, (hg + 1) * NG)
            wqT = wq_p[hs_q].reshape(NH * D, HID).T          # [HID, 1024]
            wqT_t = np.ascontiguousarray(
                wqT.reshape(KT, P, NH, P).transpose(2, 0, 1, 3)).astype(bf)
            wkT = wk_p[hs_kv].reshape(NG * D, HID).T         # [HID, 256]
            wkT_t = np.ascontiguousarray(
                wkT.reshape(KT, P, NG, P).transpose(2, 0, 1, 3)).astype(bf)
            wvT = wv_r[hs_kv].reshape(NG * D, HID).T         # [HID, 256]
            wvT_t = np.ascontiguousarray(
                wvT.reshape(KT, P, NG * P)).astype(bf)
            woT = np.ascontiguousarray(
                wo.astype(f)[:, hg * NH * D:(hg + 1) * NH * D].T
                .reshape(NH, P, HID)).astype(bf)
            ckT = np.ascontiguousarray(
                cache_k[b].astype(f)[:, hs_kv][:, :, _PERM].transpose(1, 2, 0))
            cvP = np.ascontiguousarray(
                cache_v[b].astype(f)[:, hs_kv]
                .reshape(C // P, P, NG * P).transpose(1, 0, 2)).astype(bf)
            in_maps.append({
                "hsT": hsT, "wqT": wqT_t, "wkT": wkT_t, "wvT": wvT_t,
                "woT": woT, "ckT": ckT, "cvP": cvP,
                "cs_cc": cs_cc, "cs_pm": cs_pm,
            })
    return in_maps


def gather_output(results):
    """Sum the 2 TP partials per batch -> full [B, S, HID] output."""
    out = np.empty((B, S, HID), np.float32)
    for b in range(B):
        out[b] = results[2 * b]["y"] + results[2 * b + 1]["y"]
    return out


def kernel(hidden_states, freqs_cos, freqs_sin, attention_mask,
           cache_k, cache_v, wq, wk, wv, wo):
    # attention_mask is all-zeros by construction (see spec) - unused.
    from concourse.bass_utils import run_bass_kernel_spmd

    nc = build_bass(unroll=1)
    in_maps = prep_inputs(
        np.asarray(hidden_states), np.asarray(freqs_cos), np.asarray(freqs_sin),
        np.asarray(cache_k), np.asarray(cache_v),
        np.asarray(wq), np.asarray(wk), np.asarray(wv), np.asarray(wo))
    res = run_bass_kernel_spmd(nc, in_maps, core_ids=list(range(N_CORES)))
    return gather_output(res.results)
